# revision 25
# baseline (speedup 1.0000x reference)
"""BitSelfAttention (relative_key_query position bias) on 8 trn2 cores.

Sharding: core c -> batch b=c//2, head-group g=c%2 (8 heads of 64 dims).
Per core: q/k/v projections for its 512 output dims, then per-head
attention with the Toeplitz relative-position bias realized via a DRAM
round-trip (skewed access patterns) for the Eq/Ek tables.

v2 layout/schedule:
 - scores computed TRANSPOSED (scoresT[r, l]); softmax denominators via a
   ones-column in the PV stationary; PV uses v as the stationary operand
   (M=65) producing ctxT[d, l]; output DRAM tensor is [E, S] bf16 and the
   host transposes/casts during gather.
 - heads processed in PAIRS (partitions 0-63 / 64-127 of the same qT/kT
   tile) with interleaved emission so the K=64 band and QK matmuls pack
   into disjoint PE row-groups and run concurrently.
 - rel bias injected with a DVE add (scalar_tensor_tensor) instead of
   identity matmuls; attention mask folded into the exp bias; projection
   biases folded into the PSUM->SBUF copies (per-partition bias APs).
 - software-pipelined emission: bands of pair t+1 are emitted between the
   scores and the (deferred) divide of pair t so the stage-write/skew DMA
   chain of each pair is covered by the compute of the previous one.
"""
import math
from contextlib import ExitStack

import numpy as np

import concourse.bass as bass
import concourse.bacc as bacc
import concourse.tile as tile
from concourse import mybir
from concourse.bass_utils import run_bass_kernel_spmd

B, S, D, H = 4, 1024, 1024, 16
HD = 64
E = 512          # output dims per core (8 heads)
NHC = 8          # heads per core
WD = 2176        # scratch DRAM row width
F32 = mybir.dt.float32
BF16 = mybir.dt.bfloat16
FP8 = mybir.dt.float8e4
ADD = mybir.AluOpType.add
MULT = mybir.AluOpType.mult
EXP = mybir.ActivationFunctionType.Exp


def build_nc():
    nc = bacc.Bacc()
    hT = nc.declare_dram_parameter("hT", [D, S], BF16, isOutput=False)
    wqT = nc.declare_dram_parameter("wqT", [D, E], BF16, isOutput=False)
    wkT = nc.declare_dram_parameter("wkT", [D, E], BF16, isOutput=False)
    wvT = nc.declare_dram_parameter("wvT", [D, E], BF16, isOutput=False)
    bqT = nc.declare_dram_parameter("bqT", [128, 4], F32, isOutput=False)
    bkT = nc.declare_dram_parameter("bkT", [128, 4], F32, isOutput=False)
    bvr = nc.declare_dram_parameter("bvr", [1, E], BF16, isOutput=False)
    # deT duplicated onto partitions 64..127 so odd heads' K=64 matmuls
    # use lhsT and rhs at the same base partition.
    deTR = nc.declare_dram_parameter("deTR", [128, WD], BF16, isOutput=False)
    deTP = nc.declare_dram_parameter("deTP", [128, WD], BF16, isOutput=False)
    mmask = nc.declare_dram_parameter("mmask", [128, 8], F32, isOutput=False)
    ident = nc.declare_dram_parameter("ident", [128, 128], BF16, isOutput=False)
    out_t = nc.declare_dram_parameter("out", [E, S], BF16, isOutput=True)

    # DRAM scratch, fresh per head (no WAR fan-in on reuse)
    eqr = [nc.dram_tensor(f"eqr{i}", [S, WD], BF16) for i in range(8)]
    ekd = [nc.dram_tensor(f"ekd{i}", [S, WD], FP8) for i in range(8)]

    ctx = ExitStack()
    with ctx:
        tc = ctx.enter_context(tile.TileContext(nc))
        pers = ctx.enter_context(tc.tile_pool(name="pers", bufs=1))
        # PSUM: ringA 3 x [128,1024] f32 = 6 banks; ringB 2 x [128,512] = 2
        ringA = ctx.enter_context(tc.tile_pool(name="ringA", bufs=3, space="PSUM"))
        ringB = ctx.enter_context(tc.tile_pool(name="ringB", bufs=2, space="PSUM"))
        proj_es = ExitStack()
        proj_pool = proj_es.enter_context(tc.tile_pool(name="proj", bufs=1))
        pools = {}

        # ---- load inputs to SBUF ----
        wq_sb, wk_sb, wv_sb = [], [], []
        for kt in range(8):
            t = proj_pool.tile([128, E], BF16, name=f"wq{kt}")
            nc.sync.dma_start(out=t, in_=wqT[kt * 128:(kt + 1) * 128, :])
            wq_sb.append(t)
        ht_sb = []
        for kt in range(8):
            t = proj_pool.tile([128, S], BF16, name=f"ht{kt}")
            nc.sync.dma_start(out=t, in_=hT[kt * 128:(kt + 1) * 128, :])
            ht_sb.append(t)
        for kt in range(8):
            t = proj_pool.tile([128, E], BF16, name=f"wk{kt}")
            nc.sync.dma_start(out=t, in_=wkT[kt * 128:(kt + 1) * 128, :])
            wk_sb.append(t)
        bq_sb = pers.tile([128, 4], F32, name="bq_sb")
        nc.sync.dma_start(out=bq_sb, in_=bqT[:, :])
        bk_sb = pers.tile([128, 4], F32, name="bk_sb")
        nc.sync.dma_start(out=bk_sb, in_=bkT[:, :])
        detr_sb = pers.tile([128, WD], BF16, name="detr_sb")
        nc.sync.dma_start(out=detr_sb, in_=deTR[:, :])
        detp_sb = pers.tile([128, WD], BF16, name="detp_sb")
        nc.sync.dma_start(out=detp_sb, in_=deTP[:, :])
        for kt in range(8):
            t = proj_pool.tile([128, E], BF16, name=f"wv{kt}")
            nc.sync.dma_start(out=t, in_=wvT[kt * 128:(kt + 1) * 128, :])
            wv_sb.append(t)
        bv_sb = pers.tile([1, E], BF16, name="bv_sb")
        nc.sync.dma_start(out=bv_sb, in_=bvr[:, :])
        mask_sb = pers.tile([128, 8], F32, name="mask_sb")
        nc.sync.dma_start(out=mask_sb, in_=mmask[:, :])
        id_sb = pers.tile([128, 128], BF16, name="id_sb")
        nc.sync.dma_start(out=id_sb, in_=ident[:, :])
        ones_sb = pers.tile([1, 128], BF16, name="ones_sb")
        nc.vector.memset(ones_sb, 1.0)

        qT_sb = [pers.tile([128, S], BF16, name=f"qT{et}") for et in range(4)]
        kT_sb = [pers.tile([128, S], BF16, name=f"kT{et}") for et in range(4)]
        v_sb = [pers.tile([128, 8, 65], BF16, name=f"v{st}") for st in range(8)]

        cp_tog = [0]

        def copy_op(dst, src):
            cp_tog[0] ^= 1
            if cp_tog[0]:
                nc.vector.tensor_scalar_add(dst, src, 0.0)
            else:
                nc.scalar.copy(dst, src)

        def proj_qk(et):
            for (w_sb, b_sb, dstl) in ((wq_sb, bq_sb, qT_sb), (wk_sb, bk_sb, kT_sb)):
                ps = ringA.tile([128, 1024], F32, name="ps_proj", tag="A")
                for ns in range(2):
                    psv = ps[:, ns * 512:(ns + 1) * 512]
                    for kt in range(8):
                        nc.tensor.matmul(
                            psv, w_sb[kt][:, et * 128:(et + 1) * 128],
                            ht_sb[kt][:, ns * 512:(ns + 1) * 512],
                            start=(kt == 0), stop=(kt == 7))
                bias = b_sb[:, et:et + 1]
                cp_tog[0] ^= 1
                if cp_tog[0]:
                    nc.vector.tensor_scalar_add(dstl[et], ps, bias)
                else:
                    nc.scalar.add(dstl[et], ps, bias)

        def proj_v():
            for st in range(8):
                ps = ringB.tile([128, 512], F32, name="ps_v", tag="B")
                for kt in range(8):
                    nc.tensor.matmul(
                        ps, ht_sb[kt][:, st * 128:(st + 1) * 128],
                        wv_sb[kt], start=(kt == 0), stop=False)
                nc.tensor.matmul(ps, ones_sb[0:1, 0:128], bv_sb,
                                 start=False, stop=True)
                nc.vector.tensor_copy(
                    v_sb[st][:, :, 1:65], ps.rearrange("p (h e) -> p h e", h=8))
                nc.vector.memset(v_sb[st][:, :, 0:1], 1.0)

        # per-pair state carried between emission phases
        act_T = []       # deferred ACT-ring transposes: (rel, h, r0)
        rel_tiles = {}   # (h, rt) -> tile
        ek_tiles = {}    # (h, rt) -> tile
        ex_tiles = {}    # (h, rt) -> tile
        cxs_tiles = {}   # h -> tile

        def bands(t):
            et = t
            heads = (2 * t, 2 * t + 1)
            for (nm, de_sb, drl) in (("eq", detr_sb, eqr), ("ek", detp_sb, ekd)):
                src = qT_sb[et] if nm == "eq" else kT_sb[et]
                for half in range(2):
                    stg = {}
                    sdt = BF16 if nm == "eq" else FP8
                    for h in heads:
                        stg[h] = pools['stage'].tile(
                            [128, 4, 1152], sdt, name=f"stg_{nm}{h}", tag="stage")
                    for ii in range(4):
                        i = 4 * half + ii
                        base = 896 - 128 * i
                        bA, bL = {}, {}
                        for h in heads:
                            bA[h] = ringA.tile([128, 1024], F32, name="bA", tag="A")
                            bL[h] = ringB.tile([128, 128], F32, name="bL", tag="B")
                        for wo in (0, 512):
                            for h in heads:
                                po = 64 * (h % 2)
                                nc.tensor.matmul(
                                    bA[h][:, wo:wo + 512],
                                    src[po:po + 64, i * 128:(i + 1) * 128],
                                    de_sb[po:po + 64, base + wo:base + wo + 512],
                                    start=True, stop=True)
                        for h in heads:
                            po = 64 * (h % 2)
                            nc.tensor.matmul(
                                bL[h], src[po:po + 64, i * 128:(i + 1) * 128],
                                de_sb[po:po + 64, base + 1024:base + 1152],
                                start=True, stop=True)
                        for h in heads:
                            cp_tog[0] ^= 1
                            if cp_tog[0]:
                                nc.scalar.copy(stg[h][:, ii, 0:1024], bA[h])
                                nc.vector.tensor_scalar_add(
                                    stg[h][:, ii, 1024:1152], bL[h], 0.0)
                            else:
                                nc.vector.tensor_scalar_add(
                                    stg[h][:, ii, 0:1024], bA[h], 0.0)
                                nc.scalar.copy(stg[h][:, ii, 1024:1152], bL[h])
                    weng = nc.sync if nm == "eq" else nc.gpsimd
                    for h in heads:
                        weng.dma_start(
                            out=bass.AP(
                                tensor=drl[h],
                                offset=896 + 4 * half * (128 * WD - 128),
                                ap=[[WD, 128], [128 * WD - 128, 4], [1, 1152]]),
                            in_=stg[h])
            # skew reads: Eq via xbar transpose (added on DVE later); Ek via
            # plain SWDGE read (injected into score PSUM via identity matmul).
            # No DMA-accum: an SBUF-RMW accum DMA serializes against every
            # in-flight xbar transpose (HW deadlock guard) at ~9us per link.
            for h in heads:
                for rt in range(8):
                    r0 = rt * 128
                    rel = pools['rel'].tile([128, S], BF16, name="rel", tag="rel")
                    rel_tiles[(h, rt)] = rel
                    if rt % 2 == 0:
                        nc.sync.dma_start_transpose(
                            out=rel,
                            in_=bass.AP(tensor=eqr[h], offset=1023 + r0,
                                        ap=[[WD - 1, 1024], [1, 128]]))
                    else:
                        act_T.append((rel, h, r0))
                for rt in range(8):
                    r0 = rt * 128
                    ek = pools['ek'].tile([128, S], BF16, name="ekt", tag="ek")
                    nc.gpsimd.dma_start(
                        out=ek,
                        in_=bass.AP(tensor=ekd[h], offset=(WD - 1) * r0 + 1023,
                                    ap=[[WD - 1, 128], [1, 1024]]))
                    ek_tiles[(h, rt)] = ek

        def flush_act_T(n):
            for _ in range(min(n, len(act_T))):
                rel, h, r0 = act_T.pop(0)
                nc.scalar.dma_start_transpose(
                    out=rel,
                    in_=bass.AP(tensor=eqr[h], offset=1023 + r0,
                                ap=[[WD - 1, 1024], [1, 128]]))

        def scores(t):
            et = t
            heads = (2 * t, 2 * t + 1)
            for rt in range(8):
                flush_act_T(2)
                r0 = rt * 128
                sc = {}
                for h in heads:
                    sc[h] = ringA.tile([128, 1024], F32, name="sc", tag="A")
                for nh in range(2):
                    for h in heads:
                        po = 64 * (h % 2)
                        nc.tensor.matmul(
                            sc[h][:, nh * 512:(nh + 1) * 512],
                            kT_sb[et][po:po + 64, r0:r0 + 128],
                            qT_sb[et][po:po + 64, nh * 512:(nh + 1) * 512],
                            start=True, stop=True)
                for h in heads:
                    ek = ek_tiles.pop((h, rt))
                    rel = rel_tiles.pop((h, rt))
                    # rel+ek first: all-bf16 runs at 2x DVE rate and the
                    # operands are small-magnitude, so rounding is negligible
                    nc.vector.scalar_tensor_tensor(
                        out=rel, in0=rel, scalar=0.0, in1=ek,
                        op0=ADD, op1=ADD)
                    pre = pools['pre'].tile([128, S], BF16, name="pre", tag="pre", bufs=3)
                    nc.vector.scalar_tensor_tensor(
                        out=pre, in0=sc[h], scalar=0.0, in1=rel,
                        op0=ADD, op1=ADD)
                    ex = pools['expt'].tile([128, S], BF16, name="ex", tag="ex")
                    nc.scalar.activation(
                        out=ex, in_=pre, func=EXP,
                        bias=mask_sb[:, rt:rt + 1], scale=0.125)
                    ex_tiles[(h, rt)] = ex

        def pv_mm(t):
            heads = (2 * t, 2 * t + 1)
            for h in heads:
                cxs = pools['epi'].tile([65, 1024], F32, name="cxs", tag="cxs")
                for nh in range(2):
                    cx = ringB.tile([65, 512], F32, name="cx", tag="B")
                    for rt in range(8):
                        nc.tensor.matmul(
                            cx, v_sb[rt][:, h, 0:65],
                            ex_tiles[(h, rt)][:, nh * 512:(nh + 1) * 512],
                            start=(rt == 0), stop=(rt == 7))
                    nc.vector.tensor_scalar_add(cxs[:, nh * 512:(nh + 1) * 512], cx, 0.0)
                for rt in range(8):
                    del ex_tiles[(h, rt)]
                cxs_tiles[h] = cxs

        def divide(t):
            heads = (2 * t, 2 * t + 1)
            for h in heads:
                cxs = cxs_tiles.pop(h)
                # ctxT row 0 is the softmax denominator (ones col 0 of v)
                rcb = pools['epi'].tile([65, 1024], F32, name="rcb", tag="rcb")
                nc.gpsimd.partition_broadcast(rcb, cxs[0:1, :])
                nc.vector.reciprocal_approx_fast(out=rcb, in_=rcb)
                outb = pools['epi'].tile([65, 1024], BF16, name="outb", tag="outb")
                nc.vector.scalar_tensor_tensor(
                    out=outb, in0=cxs, scalar=0.0, in1=rcb,
                    op0=ADD, op1=MULT)
                nc.gpsimd.dma_start(out=out_t[64 * h:64 * h + 64, :], in_=outb[1:65, :])

        # ---- emission schedule (software pipeline) ----
        proj_qk(0)
        proj_qk(1)
        proj_qk(2)
        proj_qk(3)
        proj_v()
        proj_es.close()
        pools['stage'] = ctx.enter_context(tc.tile_pool(name="stages", bufs=4))
        pools['rel'] = ctx.enter_context(tc.tile_pool(name="rels", bufs=20))
        pools['ek'] = ctx.enter_context(tc.tile_pool(name="eks", bufs=16))
        pools['pre'] = ctx.enter_context(tc.tile_pool(name="pres", bufs=4))
        pools['expt'] = ctx.enter_context(tc.tile_pool(name="expts", bufs=16))
        pools['epi'] = ctx.enter_context(tc.tile_pool(name="epi", bufs=2))
        bands(0)
        flush_act_T(8)
        bands(1)
        for t in range(4):
            scores(t)
            if t + 2 <= 3:
                bands(t + 2)
            pv_mm(t)
            divide(t)
    nc.compile()
    return nc


_NC_CACHE = {}
LAST_RESULT = None


def kernel(hidden_states, attention_mask, Wq, bq, Wk, bk, Wv, bv, dist_emb):
    hidden_states = np.asarray(hidden_states, np.float32)
    attention_mask = np.asarray(attention_mask, np.float32)
    Wq, bq = np.asarray(Wq, np.float32), np.asarray(bq, np.float32)
    Wk, bk = np.asarray(Wk, np.float32), np.asarray(bk, np.float32)
    Wv, bv = np.asarray(Wv, np.float32), np.asarray(bv, np.float32)
    dist_emb = np.asarray(dist_emb, np.float32)
    bf = mybir.dt.np(BF16)

    deT = dist_emb.T  # [64, 2047]
    deTP = np.zeros((128, WD), np.float32)
    deTP[0:64, :2047] = deT
    deTP[64:128, :2047] = deT
    deTR = np.zeros((128, WD), np.float32)
    deTR[0:64, :2047] = deT[:, ::-1]
    deTR[64:128, :2047] = deT[:, ::-1]

    if "nc" not in _NC_CACHE:
        _NC_CACHE["nc"] = build_nc()
    nc = _NC_CACHE["nc"]

    in_maps = []
    for c in range(8):
        b, g = c // 2, c % 2
        esl = slice(g * E, (g + 1) * E)
        in_maps.append({
            "hT": np.ascontiguousarray(hidden_states[b].T).astype(bf),
            "wqT": np.ascontiguousarray(Wq[esl, :].T).astype(bf),
            "wkT": np.ascontiguousarray(Wk[esl, :].T).astype(bf),
            "wvT": np.ascontiguousarray(Wv[esl, :].T).astype(bf),
            "bqT": np.ascontiguousarray(bq[esl].reshape(4, 128).T).astype(np.float32),
            "bkT": np.ascontiguousarray(bk[esl].reshape(4, 128).T).astype(np.float32),
            "bvr": np.ascontiguousarray(bv[esl][None, :]).astype(bf),
            "deTR": deTR.astype(bf), "deTP": deTP.astype(bf),
            "mmask": np.ascontiguousarray(
                attention_mask[b, 0, 0, :].reshape(8, 128).T).astype(np.float32),
            "ident": np.eye(128, dtype=np.float32).astype(bf),
        })
    import os as _os
    res = run_bass_kernel_spmd(nc, in_maps, core_ids=list(range(8)),
                               trace=bool(_os.environ.get("KTRACE")),
                               tmpdir=_os.environ.get("KTRACE_DIR") or None)
    global LAST_RESULT
    LAST_RESULT = res
    out = np.empty((B, S, D), np.float32)
    for c in range(8):
        b, g = c // 2, c % 2
        out[b, :, g * E:(g + 1) * E] = res.results[c]["out"].astype(np.float32).T
    return out


# revision 26
# speedup vs baseline: 1.0390x; 1.0390x over previous
"""BitSelfAttention (relative_key_query position bias) on 8 trn2 cores.

Sharding: core c -> batch b=c//2, head-group g=c%2 (8 heads of 64 dims).
Per core: q/k/v projections for its 512 output dims, then per-head
attention with the Toeplitz relative-position bias realized via a DRAM
round-trip (skewed access patterns) for the Eq/Ek tables.

v2 layout/schedule:
 - scores computed TRANSPOSED (scoresT[r, l]); softmax denominators via a
   ones-column in the PV stationary; PV uses v as the stationary operand
   (M=65) producing ctxT[d, l]; output DRAM tensor is [E, S] bf16 and the
   host transposes/casts during gather.
 - heads processed in PAIRS (partitions 0-63 / 64-127 of the same qT/kT
   tile) with interleaved emission so the K=64 band and QK matmuls pack
   into disjoint PE row-groups and run concurrently.
 - rel bias injected with a DVE add (scalar_tensor_tensor) instead of
   identity matmuls; attention mask folded into the exp bias; projection
   biases folded into the PSUM->SBUF copies (per-partition bias APs).
 - software-pipelined emission: bands of pair t+1 are emitted between the
   scores and the (deferred) divide of pair t so the stage-write/skew DMA
   chain of each pair is covered by the compute of the previous one.
"""
import math
from contextlib import ExitStack

import numpy as np

import concourse.bass as bass
import concourse.bacc as bacc
import concourse.tile as tile
from concourse import mybir
from concourse.bass_utils import run_bass_kernel_spmd

B, S, D, H = 4, 1024, 1024, 16
HD = 64
E = 512          # output dims per core (8 heads)
NHC = 8          # heads per core
WD = 2176        # scratch DRAM row width
F32 = mybir.dt.float32
BF16 = mybir.dt.bfloat16
FP8 = mybir.dt.float8e4
ADD = mybir.AluOpType.add
MULT = mybir.AluOpType.mult
EXP = mybir.ActivationFunctionType.Exp


def build_nc():
    nc = bacc.Bacc()
    hT = nc.declare_dram_parameter("hT", [D, S], BF16, isOutput=False)
    wqT = nc.declare_dram_parameter("wqT", [D, E], BF16, isOutput=False)
    wkT = nc.declare_dram_parameter("wkT", [D, E], BF16, isOutput=False)
    wvT = nc.declare_dram_parameter("wvT", [D, E], BF16, isOutput=False)
    bqT = nc.declare_dram_parameter("bqT", [128, 4], F32, isOutput=False)
    bkT = nc.declare_dram_parameter("bkT", [128, 4], F32, isOutput=False)
    bvr = nc.declare_dram_parameter("bvr", [1, E], BF16, isOutput=False)
    # deT duplicated onto partitions 64..127 so odd heads' K=64 matmuls
    # use lhsT and rhs at the same base partition.
    deTR = nc.declare_dram_parameter("deTR", [128, WD], BF16, isOutput=False)
    deTP = nc.declare_dram_parameter("deTP", [128, WD], BF16, isOutput=False)
    mmask = nc.declare_dram_parameter("mmask", [128, 8], F32, isOutput=False)
    ident = nc.declare_dram_parameter("ident", [128, 128], BF16, isOutput=False)
    out_t = nc.declare_dram_parameter("out", [E, S], BF16, isOutput=True)

    # DRAM scratch, fresh per head (no WAR fan-in on reuse)
    eqr = [nc.dram_tensor(f"eqr{i}", [S, WD], BF16) for i in range(8)]
    ekd = [nc.dram_tensor(f"ekd{i}", [S, WD], FP8) for i in range(8)]

    ctx = ExitStack()
    with ctx:
        tc = ctx.enter_context(tile.TileContext(nc))
        pers = ctx.enter_context(tc.tile_pool(name="pers", bufs=1))
        # PSUM: ringA 3 x [128,1024] f32 = 6 banks; ringB 2 x [128,512] = 2
        ringA = ctx.enter_context(tc.tile_pool(name="ringA", bufs=3, space="PSUM"))
        ringB = ctx.enter_context(tc.tile_pool(name="ringB", bufs=2, space="PSUM"))
        proj_es = ExitStack()
        proj_pool = proj_es.enter_context(tc.tile_pool(name="proj", bufs=1))
        pools = {}

        # ---- load inputs to SBUF ----
        wq_sb, wk_sb, wv_sb = [], [], []
        for kt in range(8):
            t = proj_pool.tile([128, E], BF16, name=f"wq{kt}")
            nc.sync.dma_start(out=t, in_=wqT[kt * 128:(kt + 1) * 128, :])
            wq_sb.append(t)
        ht_sb = []
        for kt in range(8):
            t = proj_pool.tile([128, S], BF16, name=f"ht{kt}")
            nc.sync.dma_start(out=t, in_=hT[kt * 128:(kt + 1) * 128, :])
            ht_sb.append(t)
        for kt in range(8):
            t = proj_pool.tile([128, E], BF16, name=f"wk{kt}")
            nc.sync.dma_start(out=t, in_=wkT[kt * 128:(kt + 1) * 128, :])
            wk_sb.append(t)
        bq_sb = pers.tile([128, 4], F32, name="bq_sb")
        nc.sync.dma_start(out=bq_sb, in_=bqT[:, :])
        bk_sb = pers.tile([128, 4], F32, name="bk_sb")
        nc.sync.dma_start(out=bk_sb, in_=bkT[:, :])
        detr_sb = pers.tile([128, WD], BF16, name="detr_sb")
        nc.sync.dma_start(out=detr_sb, in_=deTR[:, :])
        detp_sb = pers.tile([128, WD], BF16, name="detp_sb")
        nc.sync.dma_start(out=detp_sb, in_=deTP[:, :])
        for kt in range(8):
            t = proj_pool.tile([128, E], BF16, name=f"wv{kt}")
            nc.sync.dma_start(out=t, in_=wvT[kt * 128:(kt + 1) * 128, :])
            wv_sb.append(t)
        bv_sb = pers.tile([1, E], BF16, name="bv_sb")
        nc.sync.dma_start(out=bv_sb, in_=bvr[:, :])
        mask_sb = pers.tile([128, 8], F32, name="mask_sb")
        nc.sync.dma_start(out=mask_sb, in_=mmask[:, :])
        id_sb = pers.tile([128, 128], BF16, name="id_sb")
        nc.sync.dma_start(out=id_sb, in_=ident[:, :])
        ones_sb = pers.tile([1, 128], BF16, name="ones_sb")
        nc.vector.memset(ones_sb, 1.0)

        qT_sb = [pers.tile([128, S], BF16, name=f"qT{et}") for et in range(4)]
        kT_sb = [pers.tile([128, S], BF16, name=f"kT{et}") for et in range(4)]
        v_sb = [pers.tile([128, 8, 65], BF16, name=f"v{st}") for st in range(8)]

        cp_tog = [0]

        def copy_op(dst, src):
            cp_tog[0] ^= 1
            if cp_tog[0]:
                nc.vector.tensor_scalar_add(dst, src, 0.0)
            else:
                nc.scalar.copy(dst, src)

        def proj_qk(et):
            for (w_sb, b_sb, dstl) in ((wq_sb, bq_sb, qT_sb), (wk_sb, bk_sb, kT_sb)):
                ps = ringA.tile([128, 1024], F32, name="ps_proj", tag="A")
                for ns in range(2):
                    psv = ps[:, ns * 512:(ns + 1) * 512]
                    for kt in range(8):
                        nc.tensor.matmul(
                            psv, w_sb[kt][:, et * 128:(et + 1) * 128],
                            ht_sb[kt][:, ns * 512:(ns + 1) * 512],
                            start=(kt == 0), stop=(kt == 7))
                bias = b_sb[:, et:et + 1]
                cp_tog[0] ^= 1
                if cp_tog[0]:
                    nc.vector.tensor_scalar_add(dstl[et], ps, bias)
                else:
                    nc.scalar.add(dstl[et], ps, bias)

        def proj_v():
            for st in range(8):
                ps = ringB.tile([128, 512], F32, name="ps_v", tag="B")
                for kt in range(8):
                    nc.tensor.matmul(
                        ps, ht_sb[kt][:, st * 128:(st + 1) * 128],
                        wv_sb[kt], start=(kt == 0), stop=False)
                nc.tensor.matmul(ps, ones_sb[0:1, 0:128], bv_sb,
                                 start=False, stop=True)
                nc.vector.tensor_copy(
                    v_sb[st][:, :, 1:65], ps.rearrange("p (h e) -> p h e", h=8))
                nc.vector.memset(v_sb[st][:, :, 0:1], 1.0)

        # per-pair state carried between emission phases
        act_T = []       # deferred ACT-ring transposes: (rel, h, r0)
        rel_tiles = {}   # (h, rt) -> tile
        ek_tiles = {}    # (h, rt) -> tile
        ex_tiles = {}    # (h, rt) -> tile
        cxs_tiles = {}   # h -> tile

        def bands(t):
            et = t
            heads = (2 * t, 2 * t + 1)
            for (nm, de_sb, drl) in (("eq", detr_sb, eqr), ("ek", detp_sb, ekd)):
                src = qT_sb[et] if nm == "eq" else kT_sb[et]
                for half in range(2):
                    stg = {}
                    sdt = BF16 if nm == "eq" else FP8
                    for h in heads:
                        stg[h] = pools['stage'].tile(
                            [128, 4, 1152], sdt, name=f"stg_{nm}{h}", tag="stage")
                    for ii in range(4):
                        i = 4 * half + ii
                        base = 896 - 128 * i
                        bA, bL = {}, {}
                        for h in heads:
                            bA[h] = ringA.tile([128, 1024], F32, name="bA", tag="A")
                            bL[h] = ringB.tile([128, 128], F32, name="bL", tag="B")
                        for wo in (0, 512):
                            for h in heads:
                                po = 64 * (h % 2)
                                nc.tensor.matmul(
                                    bA[h][:, wo:wo + 512],
                                    src[po:po + 64, i * 128:(i + 1) * 128],
                                    de_sb[po:po + 64, base + wo:base + wo + 512],
                                    start=True, stop=True)
                        for h in heads:
                            po = 64 * (h % 2)
                            nc.tensor.matmul(
                                bL[h], src[po:po + 64, i * 128:(i + 1) * 128],
                                de_sb[po:po + 64, base + 1024:base + 1152],
                                start=True, stop=True)
                        for h in heads:
                            cp_tog[0] ^= 1
                            if cp_tog[0]:
                                nc.scalar.copy(stg[h][:, ii, 0:1024], bA[h])
                                nc.vector.tensor_scalar_add(
                                    stg[h][:, ii, 1024:1152], bL[h], 0.0)
                            else:
                                nc.vector.tensor_scalar_add(
                                    stg[h][:, ii, 0:1024], bA[h], 0.0)
                                nc.scalar.copy(stg[h][:, ii, 1024:1152], bL[h])
                    for h in heads:
                        nc.sync.dma_start(
                            out=bass.AP(
                                tensor=drl[h],
                                offset=896 + 4 * half * (128 * WD - 128),
                                ap=[[WD, 128], [128 * WD - 128, 4], [1, 1152]]),
                            in_=stg[h])
            # skew reads: Eq via xbar transpose (added on DVE later); Ek via
            # plain SWDGE read (injected into score PSUM via identity matmul).
            # No DMA-accum: an SBUF-RMW accum DMA serializes against every
            # in-flight xbar transpose (HW deadlock guard) at ~9us per link.
            for h in heads:
                for rt in range(8):
                    r0 = rt * 128
                    rel = pools['rel'].tile([128, S], BF16, name="rel", tag="rel")
                    rel_tiles[(h, rt)] = rel
                    eng = nc.sync if rt % 2 == 0 else nc.scalar
                    eng.dma_start_transpose(
                        out=rel,
                        in_=bass.AP(tensor=eqr[h], offset=1023 + r0,
                                    ap=[[WD - 1, 1024], [1, 128]]))
                for rt in range(8):
                    r0 = rt * 128
                    ek = pools['ek'].tile([128, S], BF16, name="ekt", tag="ek")
                    nc.gpsimd.dma_start(
                        out=ek,
                        in_=bass.AP(tensor=ekd[h], offset=(WD - 1) * r0 + 1023,
                                    ap=[[WD - 1, 128], [1, 1024]]))
                    ek_tiles[(h, rt)] = ek

        def scores(t):
            et = t
            heads = (2 * t, 2 * t + 1)
            for rt in range(8):
                r0 = rt * 128
                sc = {}
                for h in heads:
                    sc[h] = ringA.tile([128, 1024], F32, name="sc", tag="A")
                for nh in range(2):
                    for h in heads:
                        po = 64 * (h % 2)
                        nc.tensor.matmul(
                            sc[h][:, nh * 512:(nh + 1) * 512],
                            kT_sb[et][po:po + 64, r0:r0 + 128],
                            qT_sb[et][po:po + 64, nh * 512:(nh + 1) * 512],
                            start=True, stop=True)
                for h in heads:
                    ek = ek_tiles.pop((h, rt))
                    rel = rel_tiles.pop((h, rt))
                    # rel+ek first: all-bf16 runs at 2x DVE rate and the
                    # operands are small-magnitude, so rounding is negligible
                    nc.vector.scalar_tensor_tensor(
                        out=rel, in0=rel, scalar=0.0, in1=ek,
                        op0=ADD, op1=ADD)
                    pre = pools['pre'].tile([128, S], BF16, name="pre", tag="pre", bufs=3)
                    nc.vector.scalar_tensor_tensor(
                        out=pre, in0=sc[h], scalar=0.0, in1=rel,
                        op0=ADD, op1=ADD)
                    ex = pools['expt'].tile([128, S], BF16, name="ex", tag="ex")
                    nc.scalar.activation(
                        out=ex, in_=pre, func=EXP,
                        bias=mask_sb[:, rt:rt + 1], scale=0.125)
                    ex_tiles[(h, rt)] = ex

        def pv_mm(t):
            heads = (2 * t, 2 * t + 1)
            for h in heads:
                cxs = pools['epi'].tile([65, 1024], F32, name="cxs", tag="cxs")
                for nh in range(2):
                    cx = ringB.tile([65, 512], F32, name="cx", tag="B")
                    for rt in range(8):
                        nc.tensor.matmul(
                            cx, v_sb[rt][:, h, 0:65],
                            ex_tiles[(h, rt)][:, nh * 512:(nh + 1) * 512],
                            start=(rt == 0), stop=(rt == 7))
                    nc.vector.tensor_scalar_add(cxs[:, nh * 512:(nh + 1) * 512], cx, 0.0)
                for rt in range(8):
                    del ex_tiles[(h, rt)]
                cxs_tiles[h] = cxs

        def divide(t):
            heads = (2 * t, 2 * t + 1)
            for h in heads:
                cxs = cxs_tiles.pop(h)
                # ctxT row 0 is the softmax denominator (ones col 0 of v)
                rcb = pools['epi'].tile([65, 1024], F32, name="rcb", tag="rcb")
                nc.gpsimd.partition_broadcast(rcb, cxs[0:1, :])
                nc.vector.reciprocal_approx_fast(out=rcb, in_=rcb)
                outb = pools['epi'].tile([65, 1024], BF16, name="outb", tag="outb")
                nc.vector.scalar_tensor_tensor(
                    out=outb, in0=cxs, scalar=0.0, in1=rcb,
                    op0=ADD, op1=MULT)
                nc.sync.dma_start(out=out_t[64 * h:64 * h + 64, :], in_=outb[1:65, :])

        # ---- emission schedule (software pipeline) ----
        proj_qk(0)
        proj_qk(1)
        proj_qk(2)
        proj_qk(3)
        proj_v()
        proj_es.close()
        pools['stage'] = ctx.enter_context(tc.tile_pool(name="stages", bufs=4))
        pools['rel'] = ctx.enter_context(tc.tile_pool(name="rels", bufs=20))
        pools['ek'] = ctx.enter_context(tc.tile_pool(name="eks", bufs=16))
        pools['pre'] = ctx.enter_context(tc.tile_pool(name="pres", bufs=4))
        pools['expt'] = ctx.enter_context(tc.tile_pool(name="expts", bufs=16))
        pools['epi'] = ctx.enter_context(tc.tile_pool(name="epi", bufs=2))
        bands(0)
        bands(1)
        for t in range(4):
            scores(t)
            if t + 2 <= 3:
                bands(t + 2)
            pv_mm(t)
            divide(t)
    nc.compile()
    return nc


_NC_CACHE = {}
LAST_RESULT = None


def kernel(hidden_states, attention_mask, Wq, bq, Wk, bk, Wv, bv, dist_emb):
    hidden_states = np.asarray(hidden_states, np.float32)
    attention_mask = np.asarray(attention_mask, np.float32)
    Wq, bq = np.asarray(Wq, np.float32), np.asarray(bq, np.float32)
    Wk, bk = np.asarray(Wk, np.float32), np.asarray(bk, np.float32)
    Wv, bv = np.asarray(Wv, np.float32), np.asarray(bv, np.float32)
    dist_emb = np.asarray(dist_emb, np.float32)
    bf = mybir.dt.np(BF16)

    deT = dist_emb.T  # [64, 2047]
    deTP = np.zeros((128, WD), np.float32)
    deTP[0:64, :2047] = deT
    deTP[64:128, :2047] = deT
    deTR = np.zeros((128, WD), np.float32)
    deTR[0:64, :2047] = deT[:, ::-1]
    deTR[64:128, :2047] = deT[:, ::-1]

    if "nc" not in _NC_CACHE:
        _NC_CACHE["nc"] = build_nc()
    nc = _NC_CACHE["nc"]

    in_maps = []
    for c in range(8):
        b, g = c // 2, c % 2
        esl = slice(g * E, (g + 1) * E)
        in_maps.append({
            "hT": np.ascontiguousarray(hidden_states[b].T).astype(bf),
            "wqT": np.ascontiguousarray(Wq[esl, :].T).astype(bf),
            "wkT": np.ascontiguousarray(Wk[esl, :].T).astype(bf),
            "wvT": np.ascontiguousarray(Wv[esl, :].T).astype(bf),
            "bqT": np.ascontiguousarray(bq[esl].reshape(4, 128).T).astype(np.float32),
            "bkT": np.ascontiguousarray(bk[esl].reshape(4, 128).T).astype(np.float32),
            "bvr": np.ascontiguousarray(bv[esl][None, :]).astype(bf),
            "deTR": deTR.astype(bf), "deTP": deTP.astype(bf),
            "mmask": np.ascontiguousarray(
                attention_mask[b, 0, 0, :].reshape(8, 128).T).astype(np.float32),
            "ident": np.eye(128, dtype=np.float32).astype(bf),
        })
    import os as _os
    res = run_bass_kernel_spmd(nc, in_maps, core_ids=list(range(8)),
                               trace=bool(_os.environ.get("KTRACE")),
                               tmpdir=_os.environ.get("KTRACE_DIR") or None)
    global LAST_RESULT
    LAST_RESULT = res
    out = np.empty((B, S, D), np.float32)
    for c in range(8):
        b, g = c // 2, c % 2
        out[b, :, g * E:(g + 1) * E] = res.results[c]["out"].astype(np.float32).T
    return out


# revision 29
# speedup vs baseline: 1.0828x; 1.0421x over previous
"""BitSelfAttention (relative_key_query position bias) on 8 trn2 cores.

Sharding: core c -> batch b=c//2, head-group g=c%2 (8 heads of 64 dims).
Per core: q/k/v projections for its 512 output dims, then per-head
attention with the Toeplitz relative-position bias realized via a DRAM
round-trip (skewed access patterns) for the Eq/Ek tables.

v2 layout/schedule:
 - scores computed TRANSPOSED (scoresT[r, l]); softmax denominators via a
   ones-column in the PV stationary; PV uses v as the stationary operand
   (M=65) producing ctxT[d, l]; output DRAM tensor is [E, S] bf16 and the
   host transposes/casts during gather.
 - heads processed in PAIRS (partitions 0-63 / 64-127 of the same qT/kT
   tile) with interleaved emission so the K=64 band and QK matmuls pack
   into disjoint PE row-groups and run concurrently.
 - rel bias injected with a DVE add (scalar_tensor_tensor) instead of
   identity matmuls; attention mask folded into the exp bias; projection
   biases folded into the PSUM->SBUF copies (per-partition bias APs).
 - software-pipelined emission: bands of pair t+1 are emitted between the
   scores and the (deferred) divide of pair t so the stage-write/skew DMA
   chain of each pair is covered by the compute of the previous one.
"""
import math
from contextlib import ExitStack

import numpy as np

import concourse.bass as bass
import concourse.bacc as bacc
import concourse.tile as tile
from concourse import mybir
from concourse.bass_utils import run_bass_kernel_spmd

B, S, D, H = 4, 1024, 1024, 16
HD = 64
E = 512          # output dims per core (8 heads)
NHC = 8          # heads per core
WD = 2176        # scratch DRAM row width
F32 = mybir.dt.float32
BF16 = mybir.dt.bfloat16
FP8 = mybir.dt.float8e4
ADD = mybir.AluOpType.add
MULT = mybir.AluOpType.mult
EXP = mybir.ActivationFunctionType.Exp


def build_nc():
    nc = bacc.Bacc()
    hT = nc.declare_dram_parameter("hT", [D, S], BF16, isOutput=False)
    wqT = nc.declare_dram_parameter("wqT", [D, E], BF16, isOutput=False)
    wkT = nc.declare_dram_parameter("wkT", [D, E], BF16, isOutput=False)
    wvT = nc.declare_dram_parameter("wvT", [D, E], BF16, isOutput=False)
    bqT = nc.declare_dram_parameter("bqT", [128, 4], F32, isOutput=False)
    bkT = nc.declare_dram_parameter("bkT", [128, 4], F32, isOutput=False)
    bvr = nc.declare_dram_parameter("bvr", [1, E], BF16, isOutput=False)
    # deT duplicated onto partitions 64..127 so odd heads' K=64 matmuls
    # use lhsT and rhs at the same base partition.
    deTR = nc.declare_dram_parameter("deTR", [128, WD], BF16, isOutput=False)
    deTP = nc.declare_dram_parameter("deTP", [128, WD], BF16, isOutput=False)
    mmask = nc.declare_dram_parameter("mmask", [128, 8], F32, isOutput=False)
    ident = nc.declare_dram_parameter("ident", [128, 128], BF16, isOutput=False)
    out_t = nc.declare_dram_parameter("out", [E, S], BF16, isOutput=True)

    # DRAM scratch, fresh per head (no WAR fan-in on reuse)
    eqr = [nc.dram_tensor(f"eqr{i}", [S, WD], BF16) for i in range(8)]
    ekd = [nc.dram_tensor(f"ekd{i}", [S, WD], FP8) for i in range(8)]

    ctx = ExitStack()
    with ctx:
        tc = ctx.enter_context(tile.TileContext(nc))
        pers = ctx.enter_context(tc.tile_pool(name="pers", bufs=1))
        # PSUM: ringA 3 x [128,1024] f32 = 6 banks; ringB 2 x [128,512] = 2
        ringA = ctx.enter_context(tc.tile_pool(name="ringA", bufs=3, space="PSUM"))
        ringB = ctx.enter_context(tc.tile_pool(name="ringB", bufs=2, space="PSUM"))
        proj_es = ExitStack()
        proj_pool = proj_es.enter_context(tc.tile_pool(name="proj", bufs=1))
        pools = {}

        # ---- load inputs to SBUF ----
        wq_sb, wk_sb, wv_sb = [], [], []
        for kt in range(8):
            t = proj_pool.tile([128, E], BF16, name=f"wq{kt}")
            nc.sync.dma_start(out=t, in_=wqT[kt * 128:(kt + 1) * 128, :])
            wq_sb.append(t)
        ht_sb = []
        for kt in range(8):
            t = proj_pool.tile([128, S], BF16, name=f"ht{kt}")
            nc.sync.dma_start(out=t, in_=hT[kt * 128:(kt + 1) * 128, :])
            ht_sb.append(t)
        for kt in range(8):
            t = proj_pool.tile([128, E], BF16, name=f"wk{kt}")
            nc.sync.dma_start(out=t, in_=wkT[kt * 128:(kt + 1) * 128, :])
            wk_sb.append(t)
        bq_sb = pers.tile([128, 4], F32, name="bq_sb")
        nc.sync.dma_start(out=bq_sb, in_=bqT[:, :])
        bk_sb = pers.tile([128, 4], F32, name="bk_sb")
        nc.sync.dma_start(out=bk_sb, in_=bkT[:, :])
        detr_sb = pers.tile([128, WD], BF16, name="detr_sb")
        nc.sync.dma_start(out=detr_sb, in_=deTR[:, :])
        detp_sb = pers.tile([128, WD], BF16, name="detp_sb")
        nc.sync.dma_start(out=detp_sb, in_=deTP[:, :])
        for kt in range(8):
            t = proj_pool.tile([128, E], BF16, name=f"wv{kt}")
            nc.sync.dma_start(out=t, in_=wvT[kt * 128:(kt + 1) * 128, :])
            wv_sb.append(t)
        bv_sb = pers.tile([1, E], BF16, name="bv_sb")
        nc.sync.dma_start(out=bv_sb, in_=bvr[:, :])
        mask_sb = pers.tile([128, 8], F32, name="mask_sb")
        nc.sync.dma_start(out=mask_sb, in_=mmask[:, :])
        id_sb = pers.tile([128, 128], BF16, name="id_sb")
        nc.sync.dma_start(out=id_sb, in_=ident[:, :])
        ones_sb = pers.tile([1, 128], BF16, name="ones_sb")
        nc.vector.memset(ones_sb, 1.0)

        qT_sb = [pers.tile([128, S], BF16, name=f"qT{et}") for et in range(4)]
        kT_sb = [pers.tile([128, S], BF16, name=f"kT{et}") for et in range(4)]
        v_sb = [pers.tile([128, 8, 65], BF16, name=f"v{st}") for st in range(8)]

        cp_tog = [0]

        def copy_op(dst, src):
            cp_tog[0] ^= 1
            if cp_tog[0]:
                nc.vector.tensor_scalar_add(dst, src, 0.0)
            else:
                nc.scalar.copy(dst, src)

        def proj_qk(et):
            for (w_sb, b_sb, dstl) in ((wq_sb, bq_sb, qT_sb), (wk_sb, bk_sb, kT_sb)):
                ps = ringA.tile([128, 1024], F32, name="ps_proj", tag="A")
                for ns in range(2):
                    psv = ps[:, ns * 512:(ns + 1) * 512]
                    for kt in range(8):
                        nc.tensor.matmul(
                            psv, w_sb[kt][:, et * 128:(et + 1) * 128],
                            ht_sb[kt][:, ns * 512:(ns + 1) * 512],
                            start=(kt == 0), stop=(kt == 7))
                bias = b_sb[:, et:et + 1]
                cp_tog[0] ^= 1
                if cp_tog[0]:
                    nc.vector.tensor_scalar_add(dstl[et], ps, bias)
                else:
                    nc.scalar.add(dstl[et], ps, bias)

        def proj_v():
            for st in range(8):
                ps = ringB.tile([128, 512], F32, name="ps_v", tag="B")
                for kt in range(8):
                    nc.tensor.matmul(
                        ps, ht_sb[kt][:, st * 128:(st + 1) * 128],
                        wv_sb[kt], start=(kt == 0), stop=False)
                nc.tensor.matmul(ps, ones_sb[0:1, 0:128], bv_sb,
                                 start=False, stop=True)
                nc.vector.tensor_copy(
                    v_sb[st][:, :, 1:65], ps.rearrange("p (h e) -> p h e", h=8))
                nc.vector.memset(v_sb[st][:, :, 0:1], 1.0)

        # per-pair state carried between emission phases
        act_T = []       # deferred ACT-ring transposes: (rel, h, r0)
        rel_tiles = {}   # (h, rt) -> tile
        ek_tiles = {}    # (h, rt) -> tile
        ex_tiles = {}    # (h, rt) -> tile
        cxs_tiles = {}   # h -> tile

        def bands(t):
            et = t
            heads = (2 * t, 2 * t + 1)
            for (nm, de_sb, drl) in (("eq", detr_sb, eqr), ("ek", detp_sb, ekd)):
                src = qT_sb[et] if nm == "eq" else kT_sb[et]
                for half in range(2):
                    stg = {}
                    sdt = BF16 if nm == "eq" else FP8
                    for h in heads:
                        stg[h] = pools['stage'].tile(
                            [128, 4, 1152], sdt, name=f"stg_{nm}{h}", tag="stage")
                    for ii in range(4):
                        i = 4 * half + ii
                        base = 896 - 128 * i
                        bA, bL = {}, {}
                        for h in heads:
                            bA[h] = ringA.tile([128, 1024], F32, name="bA", tag="A")
                            bL[h] = ringB.tile([128, 128], F32, name="bL", tag="B")
                        for wo in (0, 512):
                            for h in heads:
                                po = 64 * (h % 2)
                                nc.tensor.matmul(
                                    bA[h][:, wo:wo + 512],
                                    src[po:po + 64, i * 128:(i + 1) * 128],
                                    de_sb[po:po + 64, base + wo:base + wo + 512],
                                    start=True, stop=True)
                        for h in heads:
                            po = 64 * (h % 2)
                            nc.tensor.matmul(
                                bL[h], src[po:po + 64, i * 128:(i + 1) * 128],
                                de_sb[po:po + 64, base + 1024:base + 1152],
                                start=True, stop=True)
                        for h in heads:
                            cp_tog[0] ^= 1
                            if cp_tog[0]:
                                nc.scalar.copy(stg[h][:, ii, 0:1024], bA[h])
                                nc.vector.tensor_scalar_add(
                                    stg[h][:, ii, 1024:1152], bL[h], 0.0)
                            else:
                                nc.vector.tensor_scalar_add(
                                    stg[h][:, ii, 0:1024], bA[h], 0.0)
                                nc.scalar.copy(stg[h][:, ii, 1024:1152], bL[h])
                    for h in heads:
                        nc.sync.dma_start(
                            out=bass.AP(
                                tensor=drl[h],
                                offset=896 + 4 * half * (128 * WD - 128),
                                ap=[[WD, 128], [128 * WD - 128, 4], [1, 1152]]),
                            in_=stg[h])
            # skew reads: Eq via xbar transpose (added on DVE later); Ek via
            # plain SWDGE read (injected into score PSUM via identity matmul).
            # No DMA-accum: an SBUF-RMW accum DMA serializes against every
            # in-flight xbar transpose (HW deadlock guard) at ~9us per link.
            for h in heads:
                for rt in range(8):
                    r0 = rt * 128
                    rel = pools['rel'].tile([128, S], BF16, name="rel", tag="rel")
                    rel_tiles[(h, rt)] = rel
                    nc.sync.dma_start_transpose(
                        out=rel,
                        in_=bass.AP(tensor=eqr[h], offset=1023 + r0,
                                    ap=[[WD - 1, 1024], [1, 128]]))
                for rt in range(8):
                    r0 = rt * 128
                    ek = pools['ek'].tile([128, S], BF16, name="ekt", tag="ek")
                    nc.gpsimd.dma_start(
                        out=ek,
                        in_=bass.AP(tensor=ekd[h], offset=(WD - 1) * r0 + 1023,
                                    ap=[[WD - 1, 128], [1, 1024]]))
                    ek_tiles[(h, rt)] = ek

        def scores(t):
            et = t
            heads = (2 * t, 2 * t + 1)
            for rt in range(8):
                r0 = rt * 128
                sc = {}
                for h in heads:
                    sc[h] = ringA.tile([128, 1024], F32, name="sc", tag="A")
                for nh in range(2):
                    for h in heads:
                        po = 64 * (h % 2)
                        nc.tensor.matmul(
                            sc[h][:, nh * 512:(nh + 1) * 512],
                            kT_sb[et][po:po + 64, r0:r0 + 128],
                            qT_sb[et][po:po + 64, nh * 512:(nh + 1) * 512],
                            start=True, stop=True)
                for h in heads:
                    ek = ek_tiles.pop((h, rt))
                    rel = rel_tiles.pop((h, rt))
                    nc.vector.scalar_tensor_tensor(
                        out=rel, in0=rel, scalar=0.0, in1=ek,
                        op0=ADD, op1=ADD)
                    pre = pools['pre'].tile([128, S], BF16, name="pre", tag="pre", bufs=3)
                    nc.vector.scalar_tensor_tensor(
                        out=pre, in0=sc[h], scalar=0.0, in1=rel,
                        op0=ADD, op1=ADD)
                    ex = pools['expt'].tile([128, S], BF16, name="ex", tag="ex")
                    nc.scalar.activation(
                        out=ex, in_=pre, func=EXP,
                        bias=mask_sb[:, rt:rt + 1], scale=0.125)
                    ex_tiles[(h, rt)] = ex

        def pv_mm(t):
            heads = (2 * t, 2 * t + 1)
            for h in heads:
                cxs = pools['epi'].tile([65, 1024], F32, name="cxs", tag="cxs")
                for nh in range(2):
                    cx = ringB.tile([65, 512], F32, name="cx", tag="B")
                    for rt in range(8):
                        nc.tensor.matmul(
                            cx, v_sb[rt][:, h, 0:65],
                            ex_tiles[(h, rt)][:, nh * 512:(nh + 1) * 512],
                            start=(rt == 0), stop=(rt == 7))
                    nc.vector.tensor_scalar_add(cxs[:, nh * 512:(nh + 1) * 512], cx, 0.0)
                for rt in range(8):
                    del ex_tiles[(h, rt)]
                cxs_tiles[h] = cxs

        def divide(t):
            heads = (2 * t, 2 * t + 1)
            for h in heads:
                cxs = cxs_tiles.pop(h)
                # ctxT row 0 is the softmax denominator (ones col 0 of v)
                rcb = pools['epi'].tile([65, 1024], F32, name="rcb", tag="rcb")
                nc.gpsimd.partition_broadcast(rcb, cxs[0:1, :])
                nc.vector.reciprocal_approx_fast(out=rcb, in_=rcb)
                outb = pools['epi'].tile([65, 1024], BF16, name="outb", tag="outb")
                nc.vector.scalar_tensor_tensor(
                    out=outb, in0=cxs, scalar=0.0, in1=rcb,
                    op0=ADD, op1=MULT)
                nc.sync.dma_start(out=out_t[64 * h:64 * h + 64, :], in_=outb[1:65, :])

        # ---- emission schedule (software pipeline) ----
        proj_qk(0)
        proj_qk(1)
        proj_qk(2)
        proj_qk(3)
        proj_v()
        proj_es.close()
        pools['stage'] = ctx.enter_context(tc.tile_pool(name="stages", bufs=4))
        pools['rel'] = ctx.enter_context(tc.tile_pool(name="rels", bufs=20))
        pools['ek'] = ctx.enter_context(tc.tile_pool(name="eks", bufs=16))
        pools['pre'] = ctx.enter_context(tc.tile_pool(name="pres", bufs=4))
        pools['expt'] = ctx.enter_context(tc.tile_pool(name="expts", bufs=16))
        pools['epi'] = ctx.enter_context(tc.tile_pool(name="epi", bufs=2))
        bands(0)
        bands(1)
        for t in range(4):
            scores(t)
            if t + 2 <= 3:
                bands(t + 2)
            pv_mm(t)
            divide(t)
    nc.compile()
    return nc


_NC_CACHE = {}
LAST_RESULT = None


def kernel(hidden_states, attention_mask, Wq, bq, Wk, bk, Wv, bv, dist_emb):
    hidden_states = np.asarray(hidden_states, np.float32)
    attention_mask = np.asarray(attention_mask, np.float32)
    Wq, bq = np.asarray(Wq, np.float32), np.asarray(bq, np.float32)
    Wk, bk = np.asarray(Wk, np.float32), np.asarray(bk, np.float32)
    Wv, bv = np.asarray(Wv, np.float32), np.asarray(bv, np.float32)
    dist_emb = np.asarray(dist_emb, np.float32)
    bf = mybir.dt.np(BF16)

    deT = dist_emb.T  # [64, 2047]
    deTP = np.zeros((128, WD), np.float32)
    deTP[0:64, :2047] = deT
    deTP[64:128, :2047] = deT
    deTR = np.zeros((128, WD), np.float32)
    deTR[0:64, :2047] = deT[:, ::-1]
    deTR[64:128, :2047] = deT[:, ::-1]

    if "nc" not in _NC_CACHE:
        _NC_CACHE["nc"] = build_nc()
    nc = _NC_CACHE["nc"]

    in_maps = []
    for c in range(8):
        b, g = c // 2, c % 2
        esl = slice(g * E, (g + 1) * E)
        in_maps.append({
            "hT": np.ascontiguousarray(hidden_states[b].T).astype(bf),
            "wqT": np.ascontiguousarray(Wq[esl, :].T).astype(bf),
            "wkT": np.ascontiguousarray(Wk[esl, :].T).astype(bf),
            "wvT": np.ascontiguousarray(Wv[esl, :].T).astype(bf),
            "bqT": np.ascontiguousarray(bq[esl].reshape(4, 128).T).astype(np.float32),
            "bkT": np.ascontiguousarray(bk[esl].reshape(4, 128).T).astype(np.float32),
            "bvr": np.ascontiguousarray(bv[esl][None, :]).astype(bf),
            "deTR": deTR.astype(bf), "deTP": deTP.astype(bf),
            "mmask": np.ascontiguousarray(
                attention_mask[b, 0, 0, :].reshape(8, 128).T).astype(np.float32),
            "ident": np.eye(128, dtype=np.float32).astype(bf),
        })
    import os as _os
    res = run_bass_kernel_spmd(nc, in_maps, core_ids=list(range(8)),
                               trace=bool(_os.environ.get("KTRACE")),
                               tmpdir=_os.environ.get("KTRACE_DIR") or None)
    global LAST_RESULT
    LAST_RESULT = res
    out = np.empty((B, S, D), np.float32)
    for c in range(8):
        b, g = c // 2, c % 2
        out[b, :, g * E:(g + 1) * E] = res.results[c]["out"].astype(np.float32).T
    return out
